# revision 9
# baseline (speedup 1.0000x reference)
"""CodebookLoRASTELinear forward on 8 Trainium2 NeuronCores.

out = x @ (W_q + D)^T
  D   = (lora_B @ lora_A) * (alpha/rank)
  cb  = codebook / max|codebook|
  S   = exp(scale_log)                     (per [o, i//128] group)
  q   = cb[searchsorted(midpoints(cb), (W+D)/S)]
      == cb0 + sum_k d_k * ((W+D) > t_k*S)      (S > 0)
  W_q = q * S

Column-parallel sharding: W / scale / lora_B rows (out_features) are split
across the 8 cores; x and lora_A are replicated; per-core outputs are
concatenated on the host (no collectives).

Quantization runs in natural [o, i] layout (scale is a per-partition
scalar there; comparisons stay exact fp32 -- only the final big matmul is
f32r/TF32). The effective transposed weight W_eff^T is PE-transposed into a
resident SBUF tensor folded as [128 (i%128), 32 (i//128), 512 (o)] in f32r;
x tiles are PE-transposed through PSUM into the same folded layout and
cast to f32r, then f32r matmuls (1 cyc/row) accumulate out[m(128), o(512)].
"""

import numpy as np
import sys

for _p in ("/opt/trn_rl_repo",):
    if _p not in sys.path:
        sys.path.insert(0, _p)

import concourse.mybir as mybir  # noqa: E402
import concourse.tile as tile  # noqa: E402
from concourse import bacc  # noqa: E402
from concourse.bass_utils import run_bass_kernel_spmd  # noqa: E402
from concourse.masks import make_identity  # noqa: E402
from contextlib import ExitStack  # noqa: E402

N_CORES = 8
M = 8192  # 4 * 2048 tokens
I = 4096  # in_features
O = 4096  # out_features
GROUP = 128
NG = I // GROUP  # 32 groups along i
RANK = 64
ALPHA_OVER_RANK = 32.0 / 64.0
OS = O // N_CORES  # 512 out features per core
NOB = OS // 128  # 4 output row blocks per core
NIC = I // 512  # 8 i-chunks
NMB = M // 128  # 64 m blocks

F32 = mybir.dt.float32
F32R = mybir.dt.float32r

_cache = {}


def _build_program(cb0, tk, dk):
    """cb0: smallest normalized codebook entry; tk: 3 bucket thresholds;
    dk: 3 successive codebook differences. All host floats baked in."""
    nc = bacc.Bacc("TRN2", target_bir_lowering=False, debug=False)

    x_d = nc.dram_tensor("x", [M, I], F32, kind="ExternalInput").ap()
    w_d = nc.dram_tensor("w", [OS, I], F32, kind="ExternalInput").ap()
    scl_d = nc.dram_tensor("scl", [OS, NG], F32, kind="ExternalInput").ap()
    la_d = nc.dram_tensor("la", [RANK, I], F32, kind="ExternalInput").ap()
    lbt_d = nc.dram_tensor("lbt", [RANK, OS], F32, kind="ExternalInput").ap()
    out_d = nc.dram_tensor("out", [M, OS], F32, kind="ExternalOutput").ap()

    with tile.TileContext(nc) as tc, ExitStack() as ctx:
        singles = ctx.enter_context(tc.tile_pool(name="singles", bufs=1))

        ident = singles.tile([128, 128], F32)
        make_identity(nc, ident)

        # per-partition scale scalars S = exp(scale_log), natural
        # [o%128, ob, g] layout ("scl" input already holds exp values —
        # a sub-ulp host exp keeps quantization decisions aligned with
        # the reference; the on-chip ACT Exp table is ~2e-6 off, which
        # flips buckets near thresholds)
        ssc = singles.tile([128, NOB, NG], F32)  # S
        for ob in range(NOB):
            nc.sync.dma_start(
                out=ssc[:, ob, :], in_=scl_d[ob * 128 : (ob + 1) * 128, :]
            )
        c0sc = singles.tile([128, NOB, NG], F32)  # cb0 * S
        nc.vector.tensor_scalar_mul(c0sc, ssc, float(cb0))
        tsc = []  # t_k * S
        for k in range(3):
            t = singles.tile([128, NOB, NG], F32, tag=f"tsc{k}")
            nc.vector.tensor_scalar_mul(t, ssc, float(tk[k]))
            tsc.append(t)

        la_sb = singles.tile([RANK, I], F32)
        nc.sync.dma_start(out=la_sb, in_=la_d)
        lbt_sb = singles.tile([RANK, OS], F32)
        nc.sync.dma_start(out=lbt_sb, in_=lbt_d)
        # fold alpha/rank into B^T once
        nc.vector.tensor_scalar_mul(lbt_sb, lbt_sb, float(ALPHA_OVER_RANK))

        # persistent effective transposed weight, folded [i%128, g, o], f32r
        weff = singles.tile([128, NG, OS], F32R)

        # ---- phase B: lora + quantize (natural layout) -> transpose -> weff
        wload = ctx.enter_context(tc.tile_pool(name="wload", bufs=3))
        qtmp = ctx.enter_context(tc.tile_pool(name="qtmp", bufs=2))
        wq = ctx.enter_context(tc.tile_pool(name="wq", bufs=2))
        psumD = ctx.enter_context(tc.tile_pool(name="psumD", bufs=2, space="PSUM"))
        psumW = ctx.enter_context(tc.tile_pool(name="psumW", bufs=2, space="PSUM"))

        for ob in range(NOB):
            for ic in range(NIC):
                # D chunk [128 o, 512 i] in fp32 (exact quant input)
                d_ps = psumD.tile([128, 512], F32, tag="d")
                nc.tensor.matmul(
                    d_ps,
                    lhsT=lbt_sb[:, ob * 128 : (ob + 1) * 128],
                    rhs=la_sb[:, ic * 512 : (ic + 1) * 512],
                    start=True,
                    stop=True,
                )
                wn = wload.tile([128, 512], F32, tag="wn")
                nc.sync.dma_start(
                    out=wn,
                    in_=w_d[ob * 128 : (ob + 1) * 128, ic * 512 : (ic + 1) * 512],
                )
                wqn = wq.tile([128, 512], F32, tag="wq")
                for j in range(4):
                    g = ic * 4 + j
                    sl = slice(j * 128, (j + 1) * 128)
                    u = qtmp.tile([128, 128], F32, tag="u")
                    nc.vector.tensor_add(u, wn[:, sl], d_ps[:, sl])
                    a1 = qtmp.tile([128, 128], F32, tag="a1")
                    nc.vector.tensor_scalar(
                        a1, u, tsc[0][:, ob, g : g + 1], float(dk[0]),
                        op0=mybir.AluOpType.is_gt, op1=mybir.AluOpType.mult,
                    )
                    a2 = qtmp.tile([128, 128], F32, tag="a2")
                    nc.vector.tensor_scalar(
                        a2, u, tsc[1][:, ob, g : g + 1], float(dk[1]),
                        op0=mybir.AluOpType.is_gt, op1=mybir.AluOpType.mult,
                    )
                    a3 = qtmp.tile([128, 128], F32, tag="a3")
                    nc.vector.tensor_scalar(
                        a3, u, tsc[2][:, ob, g : g + 1], float(dk[2]),
                        op0=mybir.AluOpType.is_gt, op1=mybir.AluOpType.mult,
                    )
                    nc.vector.tensor_add(a1, a1, a2)
                    nc.vector.tensor_add(a1, a1, a3)
                    # m = (q - cb0)*S + cb0*S
                    nc.vector.tensor_scalar(
                        a1, a1, ssc[:, ob, g : g + 1], c0sc[:, ob, g : g + 1],
                        op0=mybir.AluOpType.mult, op1=mybir.AluOpType.add,
                    )
                    # w_eff = q*S + D
                    nc.vector.tensor_add(wqn[:, sl], a1, d_ps[:, sl])

                # transpose [o,i] -> [i,o] and round into weff (f32r)
                pt = psumW.tile([128, 4, 128], F32, tag="pt")
                for j in range(4):
                    nc.tensor.transpose(
                        pt[:, j, :], wqn[:, j * 128 : (j + 1) * 128], ident
                    )
                nc.scalar.copy(
                    weff[:, ic * 4 : (ic + 1) * 4, ob * 128 : (ob + 1) * 128], pt
                )

        # ---- phase C: stream x, transpose, matmul --------------------------
        xpool = ctx.enter_context(tc.tile_pool(name="xpool", bufs=2))
        xtpool = ctx.enter_context(tc.tile_pool(name="xtpool", bufs=2))
        opool = ctx.enter_context(tc.tile_pool(name="opool", bufs=3))
        psumT = ctx.enter_context(tc.tile_pool(name="psumT", bufs=2, space="PSUM"))
        psumO = ctx.enter_context(tc.tile_pool(name="psumO", bufs=2, space="PSUM"))

        for mb in range(NMB):
            x_t = xpool.tile([128, I], F32, tag="x")
            nc.sync.dma_start(out=x_t, in_=x_d[mb * 128 : (mb + 1) * 128, :])

            xT = xtpool.tile([128, NG, 128], F32R, tag="xT")
            for q in range(NG // 4):
                pxt = psumT.tile([128, 4, 128], F32, tag="pxt")
                for j in range(4):
                    g = q * 4 + j
                    nc.tensor.transpose(
                        pxt[:, j, :], x_t[:, g * 128 : (g + 1) * 128], ident
                    )
                # cast-copy to f32r (this is the TF32 rounding point)
                if q % 2 == 0:
                    nc.scalar.copy(xT[:, q * 4 : (q + 1) * 4, :], pxt)
                else:
                    nc.vector.tensor_copy(xT[:, q * 4 : (q + 1) * 4, :], pxt)

            p_out = psumO.tile([128, OS], F32, tag="p_out")
            for g in range(NG):
                nc.tensor.matmul(
                    p_out,
                    lhsT=xT[:, g, :],
                    rhs=weff[:, g, :],
                    start=(g == 0),
                    stop=(g == NG - 1),
                )

            o_sb = opool.tile([128, OS], F32, tag="o")
            nc.scalar.copy(o_sb, p_out)
            nc.sync.dma_start(out=out_d[mb * 128 : (mb + 1) * 128, :], in_=o_sb)

    nc.compile()
    return nc


def _get_program(cb0, tk, dk):
    key = (round(float(cb0), 9), tuple(round(float(t), 9) for t in tk),
           tuple(round(float(d), 9) for d in dk))
    if key not in _cache:
        _cache[key] = _build_program(cb0, tk, dk)
    return _cache[key]


def kernel(x, weight, scale_log, codebook, lora_A, lora_B):
    xf = np.ascontiguousarray(x.reshape(M, I), dtype=np.float32)

    cb = np.asarray(codebook, dtype=np.float64)
    cb = cb / max(float(np.max(np.abs(cb))), 1e-8)
    tk = (cb[:-1] + cb[1:]) * 0.5
    dk = np.diff(cb)

    nc = _get_program(float(cb[0]), [float(v) for v in tk], [float(v) for v in dk])

    in_maps = []
    for c in range(N_CORES):
        sl = slice(c * OS, (c + 1) * OS)
        in_maps.append({
            "x": xf,
            "w": np.ascontiguousarray(weight[sl], dtype=np.float32),
            "scl": np.exp(np.ascontiguousarray(
                scale_log.reshape(O, NG)[sl], dtype=np.float32)),
            "la": np.ascontiguousarray(lora_A, dtype=np.float32),
            "lbt": np.ascontiguousarray(lora_B[sl].T, dtype=np.float32),
        })

    res = run_bass_kernel_spmd(nc, in_maps, core_ids=list(range(N_CORES))).results
    out = np.concatenate([res[c]["out"] for c in range(N_CORES)], axis=1)
    return out.reshape(x.shape[0], x.shape[1], O)


# revision 11
# speedup vs baseline: 1.0028x; 1.0028x over previous
"""CodebookLoRASTELinear forward on 8 Trainium2 NeuronCores.

out = x @ (W_q + D)^T
  D   = (lora_B @ lora_A) * (alpha/rank)
  cb  = codebook / max|codebook|
  S   = exp(scale_log)                     (per [o, i//128] group)
  q   = cb[searchsorted(midpoints(cb), (W+D)/S)]
      == cb0 + sum_k d_k * ((W+D) > t_k*S)      (S > 0)
  W_q = q * S

Column-parallel sharding: W / scale / lora_B rows (out_features) are split
across the 8 cores; x and lora_A are replicated; per-core outputs are
concatenated on the host (no collectives).

Quantization runs in natural [o, i] layout (scale is a per-partition
scalar there; comparisons stay exact fp32 -- only the final big matmul is
f32r/TF32, which rounds operands to ~11 mantissa bits). Phase B is g-major
so the folded W_eff^T [128 (i%128), 32 (i//128), 512 (o)] fills
group-by-group and phase C's accumulation chains can start early. x tiles
are PE-transposed (f32r, 1.5 cyc/row) through PSUM and cast-copied into the
same folded layout; f32r matmuls (1 cyc/row) accumulate out[m(128), o(512)].
"""

import numpy as np
import sys

for _p in ("/opt/trn_rl_repo",):
    if _p not in sys.path:
        sys.path.insert(0, _p)

import concourse.mybir as mybir  # noqa: E402
import concourse.tile as tile  # noqa: E402
from concourse import bacc  # noqa: E402
from concourse.bass_utils import run_bass_kernel_spmd  # noqa: E402
from concourse.masks import make_identity  # noqa: E402
from contextlib import ExitStack  # noqa: E402

N_CORES = 8
M = 8192  # 4 * 2048 tokens
I = 4096  # in_features
O = 4096  # out_features
GROUP = 128
NG = I // GROUP  # 32 groups along i
RANK = 64
ALPHA_OVER_RANK = 32.0 / 64.0
OS = O // N_CORES  # 512 out features per core
NOB = OS // 128  # 4 output row blocks per core
NMB = M // 128  # 64 m blocks

F32 = mybir.dt.float32
F32R = mybir.dt.float32r

_cache = {}


def _build_program(cb0, tk, dk, reps=1):
    """cb0: smallest normalized codebook entry; tk: 3 bucket thresholds;
    dk: 3 successive codebook differences. All host floats baked in."""
    nc = bacc.Bacc("TRN2", target_bir_lowering=False, debug=False)

    x_d = nc.dram_tensor("x", [M, I], F32R, kind="ExternalInput").ap()
    w_d = nc.dram_tensor("w", [OS, I], F32, kind="ExternalInput").ap()
    scl_d = nc.dram_tensor("scl", [OS, NG], F32, kind="ExternalInput").ap()
    la_d = nc.dram_tensor("la", [RANK, I], F32, kind="ExternalInput").ap()
    lbt_d = nc.dram_tensor("lbt", [RANK, OS], F32, kind="ExternalInput").ap()
    out_d = nc.dram_tensor("out", [M, OS], F32, kind="ExternalOutput").ap()

    with tile.TileContext(nc) as tc, ExitStack() as ctx:
        singles = ctx.enter_context(tc.tile_pool(name="singles", bufs=1))

        ident = singles.tile([128, 128], F32)
        make_identity(nc, ident)
        identr = singles.tile([128, 128], F32R)
        nc.vector.tensor_copy(identr, ident)

        # per-partition scale scalars S = exp(scale_log), natural
        # [o%128, ob, g] layout ("scl" already holds exp values -- a
        # sub-ulp host exp keeps quantization decisions aligned with the
        # reference; the on-chip ACT Exp table is ~2e-6 off, which flips
        # buckets near thresholds)
        ssc = singles.tile([128, NOB, NG], F32)  # S
        for ob in range(NOB):
            nc.sync.dma_start(
                out=ssc[:, ob, :], in_=scl_d[ob * 128 : (ob + 1) * 128, :]
            )
        c0sc = singles.tile([128, NOB, NG], F32)  # cb0 * S
        nc.vector.tensor_scalar_mul(c0sc, ssc, float(cb0))
        tsc = []  # t_k * S
        for k in range(3):
            t = singles.tile([128, NOB, NG], F32, tag=f"tsc{k}")
            nc.vector.tensor_scalar_mul(t, ssc, float(tk[k]))
            tsc.append(t)

        la_sb = singles.tile([RANK, I], F32)
        nc.sync.dma_start(out=la_sb, in_=la_d)
        lbt_sb = singles.tile([RANK, OS], F32)
        nc.sync.dma_start(out=lbt_sb, in_=lbt_d)
        # fold alpha/rank into B^T once
        nc.vector.tensor_scalar_mul(lbt_sb, lbt_sb, float(ALPHA_OVER_RANK))

        # persistent effective transposed weight, folded [i%128, g, o], f32r
        weff = singles.tile([128, NG, OS], F32R)

        if reps > 1:
            ctx.enter_context(tc.For_i(0, reps, 1))

        # ---- phase B (g-major): lora + quantize -> transpose -> weff[g] ----
        wload = ctx.enter_context(tc.tile_pool(name="wload", bufs=8))
        qtmp = ctx.enter_context(tc.tile_pool(name="qtmp", bufs=3))
        wq = ctx.enter_context(tc.tile_pool(name="wq", bufs=8))
        psumD = ctx.enter_context(tc.tile_pool(name="psumD", bufs=2, space="PSUM"))
        psumW = ctx.enter_context(tc.tile_pool(name="psumW", bufs=2, space="PSUM"))

        for g in range(NG):
            gsl = slice(g * 128, (g + 1) * 128)
            # lora delta for all 4 o-blocks of this group, fp32-exact
            d_all = psumD.tile([128, NOB, 128], F32, tag="d")
            for ob in range(NOB):
                nc.tensor.matmul(
                    d_all[:, ob, :],
                    lhsT=lbt_sb[:, ob * 128 : (ob + 1) * 128],
                    rhs=la_sb[:, gsl],
                    start=True,
                    stop=True,
                )
            pt = psumW.tile([128, NOB, 128], F32R, tag="pt")
            for ob in range(NOB):
                wn = wload.tile([128, 128], F32, tag="wn")
                nc.sync.dma_start(out=wn, in_=w_d[ob * 128 : (ob + 1) * 128, gsl])
                u = qtmp.tile([128, 128], F32, tag="u")
                nc.vector.tensor_add(u, wn, d_all[:, ob, :])
                a1 = qtmp.tile([128, 128], F32, tag="a1")
                nc.vector.tensor_scalar(
                    a1, u, tsc[0][:, ob, g : g + 1], float(dk[0]),
                    op0=mybir.AluOpType.is_gt, op1=mybir.AluOpType.mult,
                )
                a2 = qtmp.tile([128, 128], F32, tag="a2")
                nc.vector.tensor_scalar(
                    a2, u, tsc[1][:, ob, g : g + 1], float(dk[1]),
                    op0=mybir.AluOpType.is_gt, op1=mybir.AluOpType.mult,
                )
                a3 = qtmp.tile([128, 128], F32, tag="a3")
                nc.vector.tensor_scalar(
                    a3, u, tsc[2][:, ob, g : g + 1], float(dk[2]),
                    op0=mybir.AluOpType.is_gt, op1=mybir.AluOpType.mult,
                )
                # staircase sum on the (otherwise idle) gpsimd engine
                nc.gpsimd.tensor_add(a1, a1, a2)
                nc.gpsimd.tensor_add(a1, a1, a3)
                # m = (q - cb0)*S + cb0*S
                nc.vector.tensor_scalar(
                    a1, a1, ssc[:, ob, g : g + 1], c0sc[:, ob, g : g + 1],
                    op0=mybir.AluOpType.mult, op1=mybir.AluOpType.add,
                )
                # w_eff = q*S + D, rounded to f32r on write
                wqn = wq.tile([128, 128], F32R, tag="wq")
                nc.vector.tensor_add(wqn, a1, d_all[:, ob, :])
                nc.tensor.transpose(pt[:, ob, :], wqn, identr)
            # one cast-copy lands the whole group row of W_eff^T
            nc.scalar.copy(weff[:, g, :], pt.bitcast(F32))

        # ---- phase C: stream x, transpose, matmul --------------------------
        xpool = ctx.enter_context(tc.tile_pool(name="xpool", bufs=2))
        xtpool = ctx.enter_context(tc.tile_pool(name="xtpool", bufs=3))
        opool = ctx.enter_context(tc.tile_pool(name="opool", bufs=3))
        psumT = ctx.enter_context(tc.tile_pool(name="psumT", bufs=2, space="PSUM"))
        psumO = ctx.enter_context(tc.tile_pool(name="psumO", bufs=2, space="PSUM"))

        for mb in range(NMB):
            x_t = xpool.tile([128, I], F32R, tag="x")
            nc.sync.dma_start(out=x_t, in_=x_d[mb * 128 : (mb + 1) * 128, :])

            xT = xtpool.tile([128, NG, 128], F32R, tag="xT")
            for q in range(NG // 4):
                pxt = psumT.tile([128, 4, 128], F32R, tag="pxt")
                for j in range(4):
                    g = q * 4 + j
                    nc.tensor.transpose(
                        pxt[:, j, :], x_t[:, g * 128 : (g + 1) * 128], identr
                    )
                # cast-copy (bitcast input so the verifier sees an f32->f32r
                # rounding op; transpose output doesn't count as rounded)
                if q % 2 == 0:
                    nc.scalar.copy(xT[:, q * 4 : (q + 1) * 4, :], pxt.bitcast(F32))
                else:
                    nc.vector.tensor_copy(xT[:, q * 4 : (q + 1) * 4, :],
                                          pxt.bitcast(F32))

            p_out = psumO.tile([128, OS], F32, tag="p_out")
            for g in range(NG):
                nc.tensor.matmul(
                    p_out,
                    lhsT=xT[:, g, :],
                    rhs=weff[:, g, :],
                    start=(g == 0),
                    stop=(g == NG - 1),
                )

            o_sb = opool.tile([128, OS], F32, tag="o")
            nc.scalar.copy(o_sb, p_out)
            nc.sync.dma_start(out=out_d[mb * 128 : (mb + 1) * 128, :], in_=o_sb)

    nc.compile()
    return nc


def _get_program(cb0, tk, dk, reps=1):
    key = (round(float(cb0), 9), tuple(round(float(t), 9) for t in tk),
           tuple(round(float(d), 9) for d in dk), reps)
    if key not in _cache:
        _cache[key] = _build_program(cb0, tk, dk, reps)
    return _cache[key]


def kernel(x, weight, scale_log, codebook, lora_A, lora_B):
    xf = np.ascontiguousarray(x.reshape(M, I), dtype=np.float32)

    cb = np.asarray(codebook, dtype=np.float64)
    cb = cb / max(float(np.max(np.abs(cb))), 1e-8)
    tk = (cb[:-1] + cb[1:]) * 0.5
    dk = np.diff(cb)

    nc = _get_program(float(cb[0]), [float(v) for v in tk], [float(v) for v in dk])

    in_maps = []
    for c in range(N_CORES):
        sl = slice(c * OS, (c + 1) * OS)
        in_maps.append({
            "x": xf,
            "w": np.ascontiguousarray(weight[sl], dtype=np.float32),
            "scl": np.exp(np.ascontiguousarray(
                scale_log.reshape(O, NG)[sl], dtype=np.float32)),
            "la": np.ascontiguousarray(lora_A, dtype=np.float32),
            "lbt": np.ascontiguousarray(lora_B[sl].T, dtype=np.float32),
        })

    res = run_bass_kernel_spmd(nc, in_maps, core_ids=list(range(N_CORES))).results
    out = np.concatenate([res[c]["out"] for c in range(N_CORES)], axis=1)
    return out.reshape(x.shape[0], x.shape[1], O)


# revision 13
# speedup vs baseline: 62.6041x; 62.4269x over previous
"""CodebookLoRASTELinear forward on 8 Trainium2 NeuronCores.

out = x @ (W_q + D)^T
  D   = (lora_B @ lora_A) * (alpha/rank)
  cb  = codebook / max|codebook|
  S   = exp(scale_log)                     (per [o, i//128] group)
  q   = cb[searchsorted(midpoints(cb), (W+D)/S)]
      == cb0 + sum_k d_k * ((W+D) > t_k*S)      (S > 0)
  W_q = q * S

Column-parallel sharding: W / scale / lora_B rows (out_features) are split
across the 8 cores; x and lora_A are replicated; per-core outputs are
concatenated on the host (no collectives).

Quantization runs in natural [o, i] layout (scale is a per-partition
scalar there; comparisons stay exact fp32 -- only the final big matmul is
f32r/TF32, which rounds operands to ~11 mantissa bits). Phase B is g-major
so the folded W_eff^T [128 (i%128), 32 (i//128), 512 (o)] fills
group-by-group and phase C's accumulation chains can start early. x tiles
are PE-transposed (f32r, 1.5 cyc/row) through PSUM and cast-copied into the
same folded layout; f32r matmuls (1 cyc/row) accumulate out[m(128), o(512)].
"""

import numpy as np
import sys

for _p in ("/opt/trn_rl_repo",):
    if _p not in sys.path:
        sys.path.insert(0, _p)

import concourse.mybir as mybir  # noqa: E402
import concourse.tile as tile  # noqa: E402
from concourse import bacc  # noqa: E402
from concourse.bass_utils import run_bass_kernel_spmd  # noqa: E402
from concourse.masks import make_identity  # noqa: E402
from contextlib import ExitStack  # noqa: E402

N_CORES = 8
M = 8192  # 4 * 2048 tokens
I = 4096  # in_features
O = 4096  # out_features
GROUP = 128
NG = I // GROUP  # 32 groups along i
RANK = 64
ALPHA_OVER_RANK = 32.0 / 64.0
OS = O // N_CORES  # 512 out features per core
NOB = OS // 128  # 4 output row blocks per core
NMB = M // 128  # 64 m blocks

F32 = mybir.dt.float32
F32R = mybir.dt.float32r

_cache = {}


def _build_program(cb0, tk, dk, reps=1):
    """cb0: smallest normalized codebook entry; tk: 3 bucket thresholds;
    dk: 3 successive codebook differences. All host floats baked in."""
    nc = bacc.Bacc("TRN2", target_bir_lowering=False, debug=False)

    x_d = nc.dram_tensor("x", [M, I], F32R, kind="ExternalInput").ap()
    w_d = nc.dram_tensor("w", [OS, I], F32, kind="ExternalInput").ap()
    scl_d = nc.dram_tensor("scl", [OS, NG], F32, kind="ExternalInput").ap()
    la_d = nc.dram_tensor("la", [RANK, I], F32, kind="ExternalInput").ap()
    lbt_d = nc.dram_tensor("lbt", [RANK, OS], F32, kind="ExternalInput").ap()
    out_d = nc.dram_tensor("out", [M, OS], F32, kind="ExternalOutput").ap()

    with tile.TileContext(nc) as tc, ExitStack() as ctx:
        singles = ctx.enter_context(tc.tile_pool(name="singles", bufs=1))

        ident = singles.tile([128, 128], F32)
        make_identity(nc, ident)
        identr = singles.tile([128, 128], F32R)
        nc.vector.tensor_copy(identr, ident)

        # per-partition scale scalars S = exp(scale_log), natural
        # [o%128, ob, g] layout ("scl" already holds exp values -- a
        # sub-ulp host exp keeps quantization decisions aligned with the
        # reference; the on-chip ACT Exp table is ~2e-6 off, which flips
        # buckets near thresholds)
        ssc = singles.tile([128, NOB, NG], F32)  # S
        for ob in range(NOB):
            nc.sync.dma_start(
                out=ssc[:, ob, :], in_=scl_d[ob * 128 : (ob + 1) * 128, :]
            )
        c0sc = singles.tile([128, NOB, NG], F32)  # cb0 * S
        nc.vector.tensor_scalar_mul(c0sc, ssc, float(cb0))
        tsc = []  # t_k * S
        for k in range(3):
            t = singles.tile([128, NOB, NG], F32, tag=f"tsc{k}")
            nc.vector.tensor_scalar_mul(t, ssc, float(tk[k]))
            tsc.append(t)

        la_sb = singles.tile([RANK, I], F32)
        nc.sync.dma_start(out=la_sb, in_=la_d)
        lbt_sb = singles.tile([RANK, OS], F32)
        nc.sync.dma_start(out=lbt_sb, in_=lbt_d)
        # fold alpha/rank into B^T once
        nc.vector.tensor_scalar_mul(lbt_sb, lbt_sb, float(ALPHA_OVER_RANK))

        # persistent effective transposed weight, folded [i%128, g, o], f32r
        weff = singles.tile([128, NG, OS], F32R)

        if reps > 1:
            ctx.enter_context(tc.For_i(0, reps, 1))

        # ---- phase B (g-major): lora + quantize -> transpose -> weff[g] ----
        wload = ctx.enter_context(tc.tile_pool(name="wload", bufs=8))
        qtmp = ctx.enter_context(tc.tile_pool(name="qtmp", bufs=3))
        wq = ctx.enter_context(tc.tile_pool(name="wq", bufs=8))
        psumD = ctx.enter_context(tc.tile_pool(name="psumD", bufs=2, space="PSUM"))
        psumW = ctx.enter_context(tc.tile_pool(name="psumW", bufs=2, space="PSUM"))

        for g in range(NG):
            gsl = slice(g * 128, (g + 1) * 128)
            # lora delta for all 4 o-blocks of this group, fp32-exact
            d_all = psumD.tile([128, NOB, 128], F32, tag="d")
            for ob in range(NOB):
                nc.tensor.matmul(
                    d_all[:, ob, :],
                    lhsT=lbt_sb[:, ob * 128 : (ob + 1) * 128],
                    rhs=la_sb[:, gsl],
                    start=True,
                    stop=True,
                )
            pt = psumW.tile([128, NOB, 128], F32R, tag="pt")
            for ob in range(NOB):
                wn = wload.tile([128, 128], F32, tag="wn")
                nc.sync.dma_start(out=wn, in_=w_d[ob * 128 : (ob + 1) * 128, gsl])
                u = qtmp.tile([128, 128], F32, tag="u")
                nc.vector.tensor_add(u, wn, d_all[:, ob, :])
                a1 = qtmp.tile([128, 128], F32, tag="a1")
                nc.vector.tensor_scalar(
                    a1, u, tsc[0][:, ob, g : g + 1], float(dk[0]),
                    op0=mybir.AluOpType.is_gt, op1=mybir.AluOpType.mult,
                )
                a2 = qtmp.tile([128, 128], F32, tag="a2")
                nc.vector.tensor_scalar(
                    a2, u, tsc[1][:, ob, g : g + 1], float(dk[1]),
                    op0=mybir.AluOpType.is_gt, op1=mybir.AluOpType.mult,
                )
                a3 = qtmp.tile([128, 128], F32, tag="a3")
                nc.vector.tensor_scalar(
                    a3, u, tsc[2][:, ob, g : g + 1], float(dk[2]),
                    op0=mybir.AluOpType.is_gt, op1=mybir.AluOpType.mult,
                )
                # staircase sum on the (otherwise idle) gpsimd engine
                nc.gpsimd.tensor_add(a1, a1, a2)
                nc.gpsimd.tensor_add(a1, a1, a3)
                # m = (q - cb0)*S + cb0*S
                nc.vector.tensor_scalar(
                    a1, a1, ssc[:, ob, g : g + 1], c0sc[:, ob, g : g + 1],
                    op0=mybir.AluOpType.mult, op1=mybir.AluOpType.add,
                )
                # w_eff = q*S + D, rounded to f32r on write
                wqn = wq.tile([128, 128], F32R, tag="wq")
                nc.vector.tensor_add(wqn, a1, d_all[:, ob, :])
                nc.tensor.transpose(pt[:, ob, :], wqn, identr)
            # one cast-copy lands the whole group row of W_eff^T
            nc.scalar.copy(weff[:, g, :], pt.bitcast(F32))

        # ---- phase C: stream x, transpose, matmul --------------------------
        xpool = ctx.enter_context(tc.tile_pool(name="xpool", bufs=2))
        xtpool = ctx.enter_context(tc.tile_pool(name="xtpool", bufs=3))
        opool = ctx.enter_context(tc.tile_pool(name="opool", bufs=3))
        psumT = ctx.enter_context(tc.tile_pool(name="psumT", bufs=2, space="PSUM"))
        psumO = ctx.enter_context(tc.tile_pool(name="psumO", bufs=2, space="PSUM"))

        for mb in range(NMB):
            x_t = xpool.tile([128, I], F32R, tag="x")
            nc.sync.dma_start(out=x_t, in_=x_d[mb * 128 : (mb + 1) * 128, :])

            xT = xtpool.tile([128, NG, 128], F32R, tag="xT")
            for q in range(NG // 4):
                pxt = psumT.tile([128, 4, 128], F32R, tag="pxt")
                for j in range(4):
                    g = q * 4 + j
                    nc.tensor.transpose(
                        pxt[:, j, :], x_t[:, g * 128 : (g + 1) * 128], identr
                    )
                # cast-copy (bitcast input so the verifier sees an f32->f32r
                # rounding op; transpose output doesn't count as rounded)
                if q % 2 == 0:
                    nc.scalar.copy(xT[:, q * 4 : (q + 1) * 4, :], pxt.bitcast(F32))
                else:
                    nc.vector.tensor_copy(xT[:, q * 4 : (q + 1) * 4, :],
                                          pxt.bitcast(F32))

            p_out = psumO.tile([128, OS], F32, tag="p_out")
            for g in range(NG):
                nc.tensor.matmul(
                    p_out,
                    lhsT=xT[:, g, :],
                    rhs=weff[:, g, :],
                    start=(g == 0),
                    stop=(g == NG - 1),
                )

            o_sb = opool.tile([128, OS], F32, tag="o")
            nc.scalar.copy(o_sb, p_out)
            nc.sync.dma_start(out=out_d[mb * 128 : (mb + 1) * 128, :], in_=o_sb)

    nc.compile()
    return nc


def _get_program(cb0, tk, dk, reps=1):
    key = (round(float(cb0), 9), tuple(round(float(t), 9) for t in tk),
           tuple(round(float(d), 9) for d in dk), reps)
    if key not in _cache:
        _cache[key] = _build_program(cb0, tk, dk, reps)
    return _cache[key]


def kernel(x, weight, scale_log, codebook, lora_A, lora_B):
    xf = np.ascontiguousarray(x.reshape(M, I), dtype=np.float32)

    cb = np.asarray(codebook, dtype=np.float64)
    cb = cb / max(float(np.max(np.abs(cb))), 1e-8)
    tk = (cb[:-1] + cb[1:]) * 0.5
    dk = np.diff(cb)

    nc = _get_program(float(cb[0]), [float(v) for v in tk], [float(v) for v in dk])

    in_maps = []
    for c in range(N_CORES):
        sl = slice(c * OS, (c + 1) * OS)
        in_maps.append({
            "x": xf,
            "w": np.ascontiguousarray(weight[sl], dtype=np.float32),
            "scl": np.exp(np.ascontiguousarray(
                scale_log.reshape(O, NG)[sl], dtype=np.float32)),
            "la": np.ascontiguousarray(lora_A, dtype=np.float32),
            "lbt": np.ascontiguousarray(lora_B[sl].T, dtype=np.float32),
        })

    res = run_bass_kernel_spmd(nc, in_maps, core_ids=list(range(N_CORES))).results
    out = np.concatenate([res[c]["out"] for c in range(N_CORES)], axis=1)
    return out.reshape(x.shape[0], x.shape[1], O)
